# revision 1
# baseline (speedup 1.0000x reference)
"""Trainium2 Bass kernel for nn_DFlashAttentionSlide (GQA attention block).

Sharding: tensor-parallel over heads across 8 NeuronCores. Core c owns
kv head c and q heads [4c, 4c+4). Activations (x/x_ctx) are replicated;
weights / kv-cache are sharded along the head dim; the output projection
is contraction-sharded, so each core returns a partial [L, HID] output
that the host sums.

Device-side layout strategy (per core):
  - projections:  q as [l, hd] (N=512 matmuls), k/v as [d, t] (N=512)
  - attention scores computed TRANSPOSED: scoresT[s, (h l)] = K @ Q^T
    with k tiles as the stationary operand and all 4 heads' q packed in
    the 512-wide moving operand.  The PV matmul consumes the exp tiles
    directly (contraction over s = partition dim) producing outT
    [d, (h l)] -- no probability transposes anywhere.
  - the causal mask is applied MULTIPLICATIVELY after exp: exp(s+m) =
    exp(s)*exp(m), with exp(mask) precomputed on the host and
    head-replicated on device, so the s-loop mask op is a flat bf16 mul.
  - RMSNorm mean-subtract is folded into the projection weights on the
    host; variance uses sum-of-squares via ones-matmul partition
    reductions; rstd broadcast back across partitions with a K=1
    ones-matmul.
  - RoPE rotate-half is a cross-partition move done with two SBUF->SBUF
    DMA copies; the sign flip is folded into host-built sin tables.
    SCALE (1/sqrt(D)) is folded into the q-side cos/sin tables.
  - all HBM traffic runs on the hardware-DGE (sync) ring; resident
    tensors (kv cache halves, mask, tables, Wo) are chunked and
    interleaved into the projection stream so the PE-feeding cT tiles
    keep queue priority.  GPSIMD does elementwise work only.
"""

import os
import sys

sys.path.insert(0, "/opt/trn_rl_repo")

import numpy as np
import ml_dtypes

import concourse.bass as bass
import concourse.bacc as bacc
import concourse.tile as tile
from concourse import mybir
from concourse.bass_utils import run_bass_kernel_spmd

BF16 = ml_dtypes.bfloat16

H, HKV, D, HALF = 32, 8, 128, 64
L, T, S, HID = 128, 1024, 4096, 4096
REP = H // HKV          # q heads per kv head (= per core)
EPS = 1e-6
SCALE = D ** -0.5
NCORES = 8
KT = HID // 128         # 32 contraction tiles for projections
ST = S // 128           # 32 s tiles for attention
SOLD = S - T            # 3072 cached stream positions kept
TNEW = T                # 1024 newly projected stream positions

FP32 = mybir.dt.float32
BF16_DT = mybir.dt.bfloat16

_PROGRAM_CACHE = {}

# Filled by kernel() when BASS_KERNEL_TRACE=1; read by test.py.
LAST_RESULTS = None


def _build_program():
    nc = bacc.Bacc("TRN2", target_bir_lowering=False, debug=False,
                   num_devices=NCORES)

    # ---- external I/O (per-core values supplied via in_maps) ----
    cT = nc.declare_dram_parameter("cT", [HID, T], BF16_DT, isOutput=False)
    wkvT = nc.declare_dram_parameter("wkvT", [HID, 256], BF16_DT, isOutput=False)
    wqT = nc.declare_dram_parameter("wqT", [HID, 512], BF16_DT, isOutput=False)
    xTp = nc.declare_dram_parameter("xTp", [128, KT * 128], BF16_DT, isOutput=False)
    woP = nc.declare_dram_parameter("woP", [128, HID // 512, REP, 512], BF16_DT, isOutput=False)
    ktold = nc.declare_dram_parameter("ktold", [D, SOLD], BF16_DT, isOutput=False)
    voldP = nc.declare_dram_parameter("voldP", [128, SOLD], BF16_DT, isOutput=False)
    identf = nc.declare_dram_parameter("identf", [128, 128], FP32, isOutput=False)
    identb2 = nc.declare_dram_parameter("identb2", [128, 128], BF16_DT, isOutput=False)
    maskT = nc.declare_dram_parameter("maskT", [128, S], BF16_DT, isOutput=False)
    cosq = nc.declare_dram_parameter("cosq", [D, L], FP32, isOutput=False)
    sinq = nc.declare_dram_parameter("sinq", [D, L], FP32, isOutput=False)
    cosk = nc.declare_dram_parameter("cosk", [D, TNEW], FP32, isOutput=False)
    sink = nc.declare_dram_parameter("sink", [D, TNEW], FP32, isOutput=False)
    qw = nc.declare_dram_parameter("qw", [D, 1], FP32, isOutput=False)
    kw = nc.declare_dram_parameter("kw", [D, 1], FP32, isOutput=False)
    y = nc.declare_dram_parameter("y", [L, HID], FP32, isOutput=True)

    with tile.TileContext(nc) as tc:
        _emit(nc, tc, cT=cT, wkvT=wkvT, wqT=wqT, xTp=xTp, woP=woP, ktold=ktold, voldP=voldP,
              identf=identf, identb2=identb2,
              maskT=maskT, cosq=cosq, sinq=sinq, cosk=cosk, sink=sink,
              qw=qw, kw=kw, y=y)
    nc.compile()
    return nc


def _emit(nc, tc, *, cT, wkvT, wqT, xTp, woP, ktold, voldP, identf, identb2,
          maskT, cosq, sinq, cosk, sink, qw, kw, y):
    from contextlib import ExitStack
    from concourse.masks import make_identity

    ctx = ExitStack()
    with ctx:
        # ---------------- pools ----------------
        consts = ctx.enter_context(tc.tile_pool(name="consts", bufs=1))
        streams = ctx.enter_context(tc.tile_pool(name="streams", bufs=1))
        proj_in = ctx.enter_context(tc.tile_pool(name="proj_in", bufs=6))
        normtmp = ctx.enter_context(tc.tile_pool(name="normtmp", bufs=1))
        sloop = ctx.enter_context(tc.tile_pool(name="sloop", bufs=4))
        psA = ctx.enter_context(tc.tile_pool(name="psA", bufs=1, space="PSUM"))
        psS = ctx.enter_context(tc.tile_pool(name="psS", bufs=3, space="PSUM"))

        # ---------------- constants (no DMA) ----------------
        ones_col = consts.tile([128, 1], FP32, tag="ones_col")
        nc.vector.memset(ones_col, 1.0)
        ones_colb = consts.tile([128, 1], BF16_DT, tag="ones_colb")
        nc.vector.memset(ones_colb, 1.0)
        ones_row = consts.tile([1, 128], FP32, tag="ones_row")
        nc.vector.memset(ones_row, 1.0)
        eps_t = consts.tile([128, 1], FP32, tag="eps")
        nc.vector.memset(eps_t, EPS)
        ident = consts.tile([128, 128], FP32, tag="ident")
        nc.sync.dma_start(ident[:], identf[:])
        identb = consts.tile([128, 128], BF16_DT, tag="identb")
        nc.sync.dma_start(identb[:], identb2[:])

        # ---------------- resident tiles (DMAs interleaved below) --------
        kts = streams.tile([128, S], BF16_DT, tag="kts")
        vt = streams.tile([128, S], BF16_DT, tag="vt")
        mask_all = streams.tile([128, S], BF16_DT, tag="mask")
        mask4 = streams.tile([128, ST, REP, 128], BF16_DT, tag="mask4")
        wo_res = streams.tile([128, HID // 512, REP, 512], BF16_DT, tag="wo")
        qw_t = consts.tile([D, 1], FP32, tag="qw")
        kw_t = consts.tile([D, 1], FP32, tag="kw")
        cosq_t = consts.tile([D, L], FP32, tag="cosq")
        sinq_t = consts.tile([D, L], FP32, tag="sinq")
        cosk_t = consts.tile([D, TNEW], FP32, tag="cosk")
        sink_t = consts.tile([D, TNEW], FP32, tag="sink")



        xT_res = streams.tile([128, KT * 128], BF16_DT, tag="xT")

        def resident_chunk(k):
            # early-needed resident loads only (kts/vt/mask/tables), split
            # across the two HWDGE queues; wo loads happen during the s-loop
            if k < 8:  # kts old: 8 x [128, 384] on qSP
                nc.sync.dma_start(kts[:, k * 384:(k + 1) * 384],
                                  ktold[:, k * 384:(k + 1) * 384])
            if 24 <= k < 32:  # x.T for the q projection: 8 x [128, 512]
                j = k - 24
                nc.sync.dma_start(xT_res[:, j * 512:(j + 1) * 512],
                                  xTp[:, j * 512:(j + 1) * 512])
            if k < 24:  # v old (host-packed): 24 x [128, 128] contiguous
                nc.scalar.dma_start(vt[:, k * 128:(k + 1) * 128],
                                    voldP[:, k * 128:(k + 1) * 128])
            if 8 <= k < 16:  # mask: 8 x [128, 512] on qSP
                j = k - 8
                nc.sync.dma_start(mask_all[:, j * 512:(j + 1) * 512],
                                  maskT[:, j * 512:(j + 1) * 512])
            if 16 <= k < 24:  # rope tables + norm weights on qSP
                j = k - 16
                if j < 2:
                    nc.sync.dma_start(cosk_t[:, j * 512:(j + 1) * 512],
                                      cosk[:, j * 512:(j + 1) * 512])
                elif j < 4:
                    jj = j - 2
                    nc.sync.dma_start(sink_t[:, jj * 512:(jj + 1) * 512],
                                      sink[:, jj * 512:(jj + 1) * 512])
                elif j == 4:
                    nc.sync.dma_start(cosq_t[:], cosq[:])
                elif j == 5:
                    nc.sync.dma_start(sinq_t[:], sinq[:])
                elif j == 6:
                    nc.sync.dma_start(qw_t[:], qw[:])
                else:
                    nc.sync.dma_start(kw_t[:], kw[:])

        # ---------------- projections ----------------
        ps_q = psA.tile([128, 512], FP32, tag="ps_q")
        ps_k0 = psA.tile([128, 512], FP32, tag="ps_k0")
        ps_k1 = psA.tile([128, 512], FP32, tag="ps_k1")
        ps_v0 = psA.tile([128, 512], FP32, tag="ps_v0")
        ps_v1 = psA.tile([128, 512], FP32, tag="ps_v1")

        with nc.named_scope("proj"):
            for k in range(KT):
                ct_k = proj_in.tile([128, T], BF16_DT, tag="ct")
                nc.sync.dma_start(ct_k[:], cT[k * 128:(k + 1) * 128, :])
                w_k = proj_in.tile([128, 256], BF16_DT, tag="wkv")
                nc.scalar.dma_start(w_k[:], wkvT[k * 128:(k + 1) * 128, :])
                resident_chunk(k)

                st = (k == 0)
                sp = (k == KT - 1)
                nc.tensor.matmul(ps_k0[:], w_k[:, 0:128], ct_k[:, 0:512],
                                 start=st, stop=sp)
                nc.tensor.matmul(ps_k1[:], w_k[:, 0:128], ct_k[:, 512:1024],
                                 start=st, stop=sp)
                nc.tensor.matmul(ps_v0[:], w_k[:, 128:256], ct_k[:, 0:512],
                                 start=st, stop=sp)
                nc.tensor.matmul(ps_v1[:], w_k[:, 128:256], ct_k[:, 512:1024],
                                 start=st, stop=sp)
            # q projection against the resident x.T (overlaps the k/v norm)
            for k in range(KT):
                wq_k = proj_in.tile([128, 512], BF16_DT, tag="wq")
                nc.scalar.dma_start(wq_k[:], wqT[k * 128:(k + 1) * 128, :])
                nc.tensor.matmul(ps_q[:], xT_res[:, k * 128:(k + 1) * 128],
                                 wq_k[:], start=(k == 0), stop=(k == KT - 1))

        # head-replicate the multiplicative mask (GPSIMD elementwise copy)
        m2d = mask_all[:].rearrange("p (s l) -> p s l", l=128)
        for r in range(REP):
            nc.vector.tensor_copy(mask4[:, :, r, :], m2d)

        with nc.named_scope("norm"):
            # copy accumulators out on ACT (idle here); frees proj banks
            qsb = normtmp.tile([128, 512], FP32, tag="qsb")
            nc.scalar.copy(qsb[:], ps_q[:])
            kc = normtmp.tile([128, TNEW], FP32, tag="kc")
            nc.scalar.copy(kc[:, 0:512], ps_k0[:])
            nc.scalar.copy(kc[:, 512:1024], ps_k1[:])
            vsb = normtmp.tile([128, TNEW], BF16_DT, tag="vsb")
            nc.scalar.copy(vsb[:, 0:512], ps_v0[:])
            nc.scalar.copy(vsb[:, 512:1024], ps_v1[:])

            # ---- q rmsnorm + rope (first: unblocks the attention loop) ----
            qsq = normtmp.tile([128, 512], FP32, tag="qsq")
            nc.vector.tensor_mul(qsq[:], qsb[:], qsb[:])
            qsos = normtmp.tile([128, REP], FP32, tag="qsos")
            nc.vector.reduce_sum(
                qsos[:],
                qsq[:].rearrange("p (h l) -> p h l", h=REP),
                axis=mybir.AxisListType.X,
            )
            qrstd = normtmp.tile([128, REP], FP32, tag="qrstd")
            nc.scalar.activation(qrstd[:], qsos[:],
                                 mybir.ActivationFunctionType.Sqrt,
                                 bias=eps_t[:], scale=1.0 / D)
            nc.vector.reciprocal(qrstd[:], qrstd[:])
            qn = normtmp.tile([128, 512], FP32, tag="qn")
            for h in range(REP):
                nc.vector.tensor_scalar_mul(qn[:, h * 128:(h + 1) * 128],
                                            qsb[:, h * 128:(h + 1) * 128],
                                            qrstd[:, h:h + 1])
            qT_all = streams.tile([128, 512], BF16_DT, tag="qT_all")
            qtw = normtmp.tile([128, 512], FP32, tag="qtw")
            for h in range(REP):
                ps_qT = psA.tile([128, 128], FP32, tag="ps_q")
                nc.tensor.transpose(ps_qT[:], qn[:, h * 128:(h + 1) * 128],
                                    ident[:])
                nc.vector.tensor_scalar_mul(qtw[:, h * 128:(h + 1) * 128],
                                            ps_qT[:], qw_t[:])
            qrot = normtmp.tile([128, 512], FP32, tag="qrot")
            nc.sync.dma_start(qrot[0:HALF, :], qtw[HALF:D, :])
            nc.sync.dma_start(qrot[HALF:D, :], qtw[0:HALF, :])
            qa = normtmp.tile([128, 512], FP32, tag="qsq")
            qb = normtmp.tile([128, 512], FP32, tag="qn")
            for h in range(REP):
                sl = slice(h * 128, (h + 1) * 128)
                nc.vector.tensor_mul(qa[:, sl], qtw[:, sl], cosq_t[:])
                nc.vector.tensor_mul(qb[:, sl], qrot[:, sl], sinq_t[:])
            nc.vector.tensor_add(qT_all[:], qa[:], qb[:])

            # ---- k rmsnorm (mean already folded into weights) + rope ----
            ksq = normtmp.tile([128, TNEW], FP32, tag="ksq")
            nc.vector.tensor_mul(ksq[:, 0:512], kc[:, 0:512], kc[:, 0:512])
            nc.vector.tensor_mul(ksq[:, 512:1024], kc[:, 512:1024],
                                 kc[:, 512:1024])
            ps_sos0 = psA.tile([1, 512], FP32, tag="ps_k0")
            ps_sos1 = psA.tile([1, 512], FP32, tag="ps_k1")
            nc.tensor.matmul(ps_sos0[:], ones_col[:], ksq[:, 0:512])
            nc.tensor.matmul(ps_sos1[:], ones_col[:], ksq[:, 512:1024])
            krstd = normtmp.tile([1, TNEW], FP32, tag="krstd")
            nc.scalar.activation(krstd[:, 0:512], ps_sos0[:],
                                 mybir.ActivationFunctionType.Sqrt,
                                 bias=eps_t[0:1, :], scale=1.0 / D)
            nc.scalar.activation(krstd[:, 512:1024], ps_sos1[:],
                                 mybir.ActivationFunctionType.Sqrt,
                                 bias=eps_t[0:1, :], scale=1.0 / D)
            nc.vector.reciprocal(krstd[:], krstd[:])
            ps_krb0 = psA.tile([128, 512], FP32, tag="ps_k0")
            ps_krb1 = psA.tile([128, 512], FP32, tag="ps_k1")
            nc.tensor.matmul(ps_krb0[:], ones_row[:], krstd[:, 0:512])
            nc.tensor.matmul(ps_krb1[:], ones_row[:], krstd[:, 512:1024])
            knw = normtmp.tile([128, TNEW], FP32, tag="knw")
            nc.vector.scalar_tensor_tensor(knw[:, 0:512], kc[:, 0:512],
                                           kw_t[:], ps_krb0[:],
                                           op0=mybir.AluOpType.mult,
                                           op1=mybir.AluOpType.mult)
            nc.vector.scalar_tensor_tensor(knw[:, 512:1024], kc[:, 512:1024],
                                           kw_t[:], ps_krb1[:],
                                           op0=mybir.AluOpType.mult,
                                           op1=mybir.AluOpType.mult)
            krot = normtmp.tile([128, TNEW], FP32, tag="krot")
            nc.sync.dma_start(krot[0:HALF, :], knw[HALF:D, :])
            nc.sync.dma_start(krot[HALF:D, :], knw[0:HALF, :])
            ka = normtmp.tile([128, TNEW], FP32, tag="ksq")
            nc.vector.tensor_mul(ka[:], knw[:], cosk_t[:])
            kb = normtmp.tile([128, TNEW], FP32, tag="kb")
            nc.vector.tensor_mul(kb[:], krot[:], sink_t[:])
            nc.vector.tensor_add(kts[:, SOLD:S], ka[:], kb[:])

            # ---- v transpose into stream tiles via PE ----
            for i in range(TNEW // 128):
                ps_vT = psA.tile([128, 128], BF16_DT, tag="ps_v1")
                nc.tensor.transpose(ps_vT[:], vsb[:, i * 128:(i + 1) * 128],
                                    identb[:])
                nc.vector.tensor_copy(
                    vt[:, SOLD + i * 128:SOLD + (i + 1) * 128], ps_vT[:])

        # ---------------- attention s-loop ----------------
        ps_o = psA.tile([128, 512], FP32, tag="ps_v0")
        ps_sum = psA.tile([1, 512], FP32, tag="ps_q")
        with nc.named_scope("sloop"):
            for s in range(ST):
                if s % 4 == 0:  # wo (host-packed): 8 contiguous 512KB chunks
                    j = s // 4
                    nc.sync.dma_start(wo_res[:, j, :, :], woP[:, j, :, :])
                ps_sc = psS.tile([128, 512], FP32, tag="ps_sc")
                nc.tensor.matmul(ps_sc[:], kts[:, s * 128:(s + 1) * 128],
                                 qT_all[:])
                scb = sloop.tile([128, 512], FP32, tag="scb")
                nc.vector.tensor_copy(scb[:], ps_sc[:])
                exr = sloop.tile([128, 512], BF16_DT, tag="exr")
                nc.scalar.activation(exr[:], scb[:],
                                     mybir.ActivationFunctionType.Exp)
                ex = sloop.tile([128, 512], BF16_DT, tag="ex")
                nc.vector.tensor_mul(
                    ex[:], exr[:],
                    mask4[:, s, :, :].rearrange("p h l -> p (h l)"))
                nc.tensor.matmul(ps_sum[:], ones_colb[:], ex[:],
                                 start=(s == 0), stop=(s == ST - 1))
                nc.tensor.matmul(ps_o[:], vt[:, s * 128:(s + 1) * 128], ex[:],
                                 start=(s == 0), stop=(s == ST - 1))

        # ---------------- normalize ----------------
        with nc.named_scope("fin"):
            rec = normtmp.tile([1, 512], FP32, tag="rec")
            nc.vector.reciprocal(rec[:], ps_sum[:])
            ps_rb = psA.tile([128, 512], FP32, tag="ps_k0")
            nc.tensor.matmul(ps_rb[:], ones_row[:], rec[:])
            osb = normtmp.tile([128, 512], FP32, tag="osb")
            nc.scalar.copy(osb[:], ps_o[:])
            attT = streams.tile([128, 512], BF16_DT, tag="attT")
            nc.vector.tensor_mul(attT[:], osb[:], ps_rb[:])

        # ---------------- output projection (partial) ----------------
        with nc.named_scope("oproj"):
            for e in range(HID // 512):
                ps_y = psA.tile([128, 512], FP32,
                                tag=("ps_k1" if e % 2 else "ps_v0"))
                for h in range(REP):
                    nc.tensor.matmul(
                        ps_y[:], attT[:, h * 128:(h + 1) * 128],
                        wo_res[:, e, h, :],
                        start=(h == 0), stop=(h == REP - 1))
                ysb = sloop.tile([128, 512], FP32, tag="ysb")
                nc.vector.tensor_copy(ysb[:], ps_y[:])
                nc.sync.dma_start(y[:, e * 512:(e + 1) * 512], ysb[:])


def _prepare_inputs(x, x_ctx, cos_q, sin_q, cos_k, sin_k, kv_cache,
                    causal_mask, Wq, Wk, Wv, Wo, q_norm_w, k_norm_w):
    """Host-side sharding/preprocessing. Returns list of per-core in_maps."""
    f32 = np.float32
    x = np.asarray(x, f32)
    x_ctx = np.asarray(x_ctx, f32)
    c = np.concatenate([x_ctx[0], x[0]], axis=0)          # [T, HID]
    cT = np.ascontiguousarray(c.T).astype(BF16)           # [HID, T]

    # x.T packed [p, (k 128l)]: xTp[p, k*128+l] = c.T[k*128+p, T-L+l]
    xTp = np.ascontiguousarray(
        c.T[:, T - L:T].reshape(KT, 128, L).transpose(1, 0, 2)
        .reshape(128, KT * L)).astype(BF16)

    m = np.asarray(causal_mask, f32)[0, 0]                # [L, S]
    # multiplicative mask exp(m), packed [s_local, (s_tile l)]
    maskP = np.ascontiguousarray(np.exp(
        m.T.reshape(S // 128, 128, L).transpose(1, 0, 2).reshape(128, S)))

    cosqT = np.ascontiguousarray(np.asarray(cos_q, f32)[0, 0].T) * SCALE
    sinqT = np.ascontiguousarray(np.asarray(sin_q, f32)[0, 0].T).copy()
    sinqT[:HALF] = -sinqT[:HALF]
    sinqT *= SCALE
    coskT = np.ascontiguousarray(np.asarray(cos_k, f32)[0, 0].T)
    sinkT = np.ascontiguousarray(np.asarray(sin_k, f32)[0, 0].T).copy()
    sinkT[:HALF] = -sinkT[:HALF]

    qwc = np.ascontiguousarray(np.asarray(q_norm_w, f32).reshape(D, 1))
    kwc = np.ascontiguousarray(np.asarray(k_norm_w, f32).reshape(D, 1))

    Wq = np.asarray(Wq, f32)
    Wk = np.asarray(Wk, f32)
    Wv = np.asarray(Wv, f32)
    Wo = np.asarray(Wo, f32)
    kv = np.asarray(kv_cache, f32)

    in_maps = []
    for cidx in range(NCORES):
        hd = slice(cidx * REP * D, (cidx + 1) * REP * D)
        wq_c = Wq[hd].reshape(REP, D, HID)
        wq_c = wq_c - wq_c.mean(axis=1, keepdims=True)    # fold mean-subtract
        wq_c = wq_c.reshape(REP * D, HID)
        wk_c = Wk[cidx * D:(cidx + 1) * D]
        wk_c = wk_c - wk_c.mean(axis=0, keepdims=True)
        wv_c = Wv[cidx * D:(cidx + 1) * D]
        wkvT = np.concatenate([wk_c.T, wv_c.T], axis=1)   # [HID, 256]
        wqTc = np.ascontiguousarray(wq_c.T)               # [HID, 512]
        # wo packed [p, e_chunk, h, 512]: woP[p,j,h,e'] = Wo.T[h*128+p, j*512+e']
        woTc = Wo[:, hd].T.reshape(REP, 128, HID // 512, 512)
        woP = np.ascontiguousarray(woTc.transpose(1, 2, 0, 3))
        ktold = np.ascontiguousarray(kv[0, cidx, T:, :].T)  # [D, SOLD]
        # vold packed [s_local, (tile d)]: voldP[p, n*128+d] = v[n*128+p, d]
        voldP = np.ascontiguousarray(
            kv[1, cidx, T:, :].reshape(SOLD // 128, 128, D)
            .transpose(1, 0, 2).reshape(128, SOLD))
        in_maps.append(dict(
            cT=cT,
            wkvT=np.ascontiguousarray(wkvT).astype(BF16),
            wqT=wqTc.astype(BF16),
            xTp=xTp,
            woP=woP.astype(BF16),
            ktold=ktold.astype(BF16),
            voldP=voldP.astype(BF16),
            identf=np.eye(128, dtype=f32),
            identb2=np.eye(128, dtype=f32).astype(BF16),
            maskT=maskP.astype(BF16),
            cosq=cosqT.astype(f32), sinq=sinqT.astype(f32),
            cosk=coskT.astype(f32), sink=sinkT.astype(f32),
            qw=qwc, kw=kwc,
        ))
    return in_maps


def kernel(**inputs) -> np.ndarray:
    global LAST_RESULTS
    if "nc" not in _PROGRAM_CACHE:
        _PROGRAM_CACHE["nc"] = _build_program()
    nc = _PROGRAM_CACHE["nc"]
    in_maps = _prepare_inputs(**inputs)
    trace = bool(int(os.environ.get("BASS_KERNEL_TRACE", "0")))
    res = run_bass_kernel_spmd(nc, in_maps, list(range(NCORES)), trace=trace)
    LAST_RESULTS = res
    y = np.zeros((L, HID), np.float64)
    for cidx in range(NCORES):
        y += res.results[cidx]["y"].astype(np.float64)
    return y.astype(np.float32).reshape(1, L, HID)



# revision 3
# speedup vs baseline: 1.4966x; 1.4966x over previous
"""Trainium2 Bass kernel for nn_DFlashAttentionSlide (GQA attention block).

Sharding: tensor-parallel over heads across 8 NeuronCores. Core c owns
kv head c and q heads [4c, 4c+4). Activations (x/x_ctx) are replicated;
weights / kv-cache are sharded along the head dim; the output projection
is contraction-sharded, so each core returns a partial [L, HID] output
that the host sums.

v2 design notes (vs the first working version):
  - Phases ordered for PE density (HAM clock gate stays warm):
    q-proj (streamed wq) -> q-norm -> kv-proj (streamed cT) ->
    old-s-tiles (with k-norm work injected on ACT/DVE/PE gaps) ->
    new-s-tiles -> softmax-normalize -> o-proj.
  - s-loop: exp reads score PSUM directly on ACT (no DVE copy); the
    causal mask only affects the last 128-token tile, so the mask
    multiply runs on that tile alone (host-verified; a full-mask
    program variant is compiled lazily if the mask is nonstandard).
  - All rsqrt computations (q/k rmsnorm) run as Newton iterations on
    DVE so ACT only ever uses {Exp, Square, Copy} = one table set,
    loaded once by a dummy activation at t=0.
  - k rmsnorm rstd is NOT applied to k; it commutes with RoPE and is
    folded into the exp() per-partition scale operand of the new
    tiles' softmax (scores rows = stream positions).
  - RoPE rotate-half is one PE matmul against a signed permutation
    matrix; sign lives in the matrix so sin tables are unmodified.
  - Host packs every stream into SBUF-layout-contiguous DRAM tensors;
    bulk traffic (wq, cT+wkv, wo) rides the qSP HWDGE ring in 0.25-1MiB
    transfers, small residents + y writeback ride qACT.
"""

import os
import sys

sys.path.insert(0, "/opt/trn_rl_repo")

import numpy as np
import ml_dtypes

import concourse.bass as bass
import concourse.bacc as bacc
import concourse.tile as tile
from concourse import mybir
from concourse.bass_utils import run_bass_kernel_spmd

BF16 = ml_dtypes.bfloat16

H, HKV, D, HALF = 32, 8, 128, 64
L, T, S, HID = 128, 1024, 4096, 4096
REP = H // HKV          # q heads per kv head (= per core)
EPS = 1e-6
SCALE = D ** -0.5
NCORES = 8
KT = HID // 128         # 32 contraction tiles for projections
ST = S // 128           # 32 s tiles for attention
SOLD = S - T            # 3072 cached stream positions kept
TNEW = T                # 1024 newly projected stream positions
NOLD = SOLD // 128      # 24 old s tiles

FP32 = mybir.dt.float32
BF16_DT = mybir.dt.bfloat16
AF = mybir.ActivationFunctionType
ALU = mybir.AluOpType

_PROGRAM_CACHE = {}

# Filled by kernel() when BASS_KERNEL_TRACE=1; read by test.py.
LAST_RESULTS = None


def _build_program(mask_all=False):
    nc = bacc.Bacc("TRN2", target_bir_lowering=False, debug=False,
                   num_devices=NCORES)

    # ---- external I/O (per-core values supplied via in_maps) ----
    wqP = nc.declare_dram_parameter("wqP", [128, KT * 512], BF16_DT, isOutput=False)
    ctP = nc.declare_dram_parameter("ctP", [128, KT * 1280], BF16_DT, isOutput=False)
    woP = nc.declare_dram_parameter("woP", [128, REP * 4096], BF16_DT, isOutput=False)
    xTp = nc.declare_dram_parameter("xTp", [128, KT * 128], BF16_DT, isOutput=False)
    ktold = nc.declare_dram_parameter("ktold", [D, SOLD], BF16_DT, isOutput=False)
    voldP = nc.declare_dram_parameter("voldP", [128, SOLD], BF16_DT, isOutput=False)
    identb2 = nc.declare_dram_parameter("identb2", [128, 128], BF16_DT, isOutput=False)
    rotTd = nc.declare_dram_parameter("rotTd", [128, 128], BF16_DT, isOutput=False)
    cosq4d = nc.declare_dram_parameter("cosq4d", [D, 512], BF16_DT, isOutput=False)
    sinq4d = nc.declare_dram_parameter("sinq4d", [D, 512], FP32, isOutput=False)
    coskd = nc.declare_dram_parameter("coskd", [D, TNEW], BF16_DT, isOutput=False)
    sinkd = nc.declare_dram_parameter("sinkd", [D, TNEW], FP32, isOutput=False)
    qwd = nc.declare_dram_parameter("qwd", [D, 1], FP32, isOutput=False)
    kwd = nc.declare_dram_parameter("kwd", [D, 1], FP32, isOutput=False)
    mask31d = nc.declare_dram_parameter("mask31d", [128, 512], BF16_DT, isOutput=False)
    maskFd = None
    if mask_all:
        maskFd = nc.declare_dram_parameter("maskFd", [128, ST * 512], BF16_DT,
                                           isOutput=False)
    y = nc.declare_dram_parameter("y", [L, HID], FP32, isOutput=True)

    with tile.TileContext(nc) as tc:
        _emit(nc, tc, mask_all=mask_all, wqP=wqP, ctP=ctP, woP=woP, xTp=xTp,
              ktold=ktold, voldP=voldP, identb2=identb2, rotTd=rotTd,
              cosq4d=cosq4d, sinq4d=sinq4d, coskd=coskd, sinkd=sinkd,
              qwd=qwd, kwd=kwd, mask31d=mask31d, maskFd=maskFd, y=y)
    nc.compile()
    return nc


def _newton_rsqrt(nc, pool, out, x_ps, n, tag):
    """out[128, n] = rsqrt(x_ps/D + EPS) via DVE-only Newton iteration.

    x_ps is the raw sum-of-squares (PSUM or SBUF). Seed is a linear fit
    valid for var ~ O(1); 5 iterations recover full fp32 accuracy over a
    wide input range, and the clamp keeps the seed positive everywhere.
    """
    xh = pool.tile([128, n], FP32, tag=tag + "_xh")
    nc.vector.tensor_scalar(xh, x_ps, 0.5 / D, 0.5 * EPS, ALU.mult, ALU.add)
    y0 = pool.tile([128, n], FP32, tag=tag + "_y")
    nc.vector.tensor_scalar(y0, x_ps, -0.235 / D, 1.2, ALU.mult, ALU.add)
    nc.vector.tensor_scalar(y0, y0, 0.03, None, ALU.max)
    t1 = pool.tile([128, n], FP32, tag=tag + "_t")
    for _ in range(5):
        nc.vector.tensor_mul(t1, xh, y0)
        nc.vector.tensor_mul(t1, t1, y0)
        nc.vector.tensor_scalar(t1, t1, -1.0, 1.5, ALU.mult, ALU.add)
        nc.vector.tensor_mul(y0, y0, t1)
    nc.vector.tensor_copy(out, y0)


def _emit(nc, tc, *, mask_all, wqP, ctP, woP, xTp, ktold, voldP, identb2,
          rotTd, cosq4d, sinq4d, coskd, sinkd, qwd, kwd, mask31d, maskFd, y):
    from contextlib import ExitStack

    ctx = ExitStack()
    with ctx:
        # ---------------- pools ----------------
        consts = ctx.enter_context(tc.tile_pool(name="consts", bufs=1))
        big = ctx.enter_context(tc.tile_pool(name="big", bufs=1))
        ctp = ctx.enter_context(tc.tile_pool(name="ctp", bufs=20))
        sloop = ctx.enter_context(tc.tile_pool(name="sloop", bufs=3))
        ypool = ctx.enter_context(tc.tile_pool(name="ypool", bufs=2))
        psA = ctx.enter_context(tc.tile_pool(name="psA", bufs=1, space="PSUM"))

        # ---------------- constants ----------------
        ones_colb = consts.tile([128, 1], BF16_DT, tag="ones_colb")
        nc.vector.memset(ones_colb, 1.0)
        ones_row = consts.tile([1, 128], FP32, tag="ones_row")
        nc.vector.memset(ones_row, 1.0)
        # warm the (single) ACT table set at t=0
        dw = consts.tile([128, 1], FP32, tag="dw")
        nc.vector.memset(dw, 0.0)
        dw2 = consts.tile([128, 1], BF16_DT, tag="dw2")
        nc.scalar.activation(dw2, dw, AF.Exp)

        # ---------------- resident tiles + early DMAs (qACT ring) -------
        xT = big.tile([128, KT * 128], BF16_DT, tag="xT")
        nc.scalar.dma_start(xT, xTp[:])
        kts_old = big.tile([128, SOLD], BF16_DT, tag="kts_old")
        nc.scalar.dma_start(kts_old, ktold[:])
        vt_old = big.tile([128, SOLD], BF16_DT, tag="vt_old")
        nc.scalar.dma_start(vt_old, voldP[:])
        identb = consts.tile([128, 128], BF16_DT, tag="identb")
        nc.scalar.dma_start(identb, identb2[:])
        rotT = consts.tile([128, 128], BF16_DT, tag="rotT")
        nc.scalar.dma_start(rotT, rotTd[:])
        cosq4 = consts.tile([D, 512], BF16_DT, tag="cosq4")
        nc.scalar.dma_start(cosq4, cosq4d[:])
        sinq4 = consts.tile([D, 512], FP32, tag="sinq4")
        nc.scalar.dma_start(sinq4, sinq4d[:])
        cosk = consts.tile([D, TNEW], BF16_DT, tag="cosk")
        nc.scalar.dma_start(cosk, coskd[:])
        sink = consts.tile([D, TNEW], FP32, tag="sink")
        nc.scalar.dma_start(sink, sinkd[:])
        qw_t = consts.tile([D, 1], FP32, tag="qw")
        nc.scalar.dma_start(qw_t, qwd[:])
        kw_t = consts.tile([D, 1], FP32, tag="kw")
        nc.scalar.dma_start(kw_t, kwd[:])
        mask31 = consts.tile([128, 512], BF16_DT, tag="mask31")
        nc.scalar.dma_start(mask31, mask31d[:])
        maskF = None
        if mask_all:
            maskF = big.tile([128, ST * 512], BF16_DT, tag="maskF")
            nc.scalar.dma_start(maskF, maskFd[:])

        # ---------------- bulk stream DMAs (qSP ring, in order) ---------
        wq = []
        for m in range(4):
            wqm = big.tile([128, 4096], BF16_DT, tag=f"wq{m}")
            nc.sync.dma_start(wqm, wqP[:, m * 4096:(m + 1) * 4096])
            wq.append(wqm)

        # ---------------- q projection (DMA-gated on wq stream) ---------
        ps_q = psA.tile([128, 512], FP32, tag="bA")
        with nc.named_scope("qproj"):
            for k in range(KT):
                nc.tensor.matmul(ps_q[:], xT[:, k * 128:(k + 1) * 128],
                                 wq[k // 8][:, (k % 8) * 512:(k % 8 + 1) * 512],
                                 start=(k == 0), stop=(k == KT - 1))

        # ct/wkv stream + wo stream (behind wq on qSP)
        cts = []
        for k in range(KT):
            ctk = ctp.tile([128, 1280], BF16_DT, tag="ct")
            nc.sync.dma_start(ctk, ctP[:, k * 1280:(k + 1) * 1280])
            cts.append(ctk)
        woR = big.tile([128, REP * 4096], BF16_DT, tag="woR")
        for h in range(REP):
            nc.sync.dma_start(woR[:, h * 4096:(h + 1) * 4096],
                              woP[:, h * 4096:(h + 1) * 4096])

        # ---------------- q norm + rope ----------------
        with nc.named_scope("qnorm"):
            qsq = big.tile([128, 512], BF16_DT, tag="qsq")
            nc.scalar.activation(qsq, ps_q[:], AF.Square)
            qsos = big.tile([128, REP], FP32, tag="qsos")
            nc.vector.reduce_sum(
                qsos, qsq[:].rearrange("p (h l) -> p h l", h=REP),
                axis=mybir.AxisListType.X)
            qrstd = big.tile([128, REP], FP32, tag="qrstd")
            _newton_rsqrt(nc, big, qrstd, qsos, REP, "qn")
            qn = big.tile([128, 512], BF16_DT, tag="qn")
            for h in range(REP):
                nc.vector.tensor_scalar_mul(qn[:, h * 128:(h + 1) * 128],
                                            ps_q[:, h * 128:(h + 1) * 128],
                                            qrstd[:, h:h + 1])
            qtw = big.tile([128, 512], BF16_DT, tag="qtw")
            for h in range(REP):
                ps_qT = psA.tile([128, 128], BF16_DT, tag="bF")
                nc.tensor.transpose(ps_qT[:], qn[:, h * 128:(h + 1) * 128],
                                    identb[:])
                nc.vector.tensor_scalar_mul(qtw[:, h * 128:(h + 1) * 128],
                                            ps_qT[:], qw_t[:])
            ps_qrot = psA.tile([128, 512], FP32, tag="bG")
            nc.tensor.matmul(ps_qrot[:], rotT[:], qtw[:])
            qa = big.tile([128, 512], BF16_DT, tag="qa")
            nc.vector.tensor_mul(qa, qtw[:], cosq4[:])
            qb = big.tile([128, 512], BF16_DT, tag="qb")
            nc.vector.tensor_mul(qb, ps_qrot[:], sinq4[:])
            qT_all = big.tile([128, 512], BF16_DT, tag="qT_all")
            nc.vector.tensor_add(qT_all, qa, qb)

        # ---------------- k/v projection (DMA-gated on ct stream) -------
        ps_k0 = psA.tile([128, 512], FP32, tag="bB")
        ps_k1 = psA.tile([128, 512], FP32, tag="bC")
        ps_v0 = psA.tile([128, 512], FP32, tag="bD")
        ps_v1 = psA.tile([128, 512], FP32, tag="bE")
        with nc.named_scope("kvproj"):
            for k in range(KT):
                ctk = cts[k]
                st = (k == 0)
                sp = (k == KT - 1)
                nc.tensor.matmul(ps_k0[:], ctk[:, 1024:1152], ctk[:, 0:512],
                                 start=st, stop=sp)
                nc.tensor.matmul(ps_k1[:], ctk[:, 1024:1152], ctk[:, 512:1024],
                                 start=st, stop=sp)
                nc.tensor.matmul(ps_v0[:], ctk[:, 1152:1280], ctk[:, 0:512],
                                 start=st, stop=sp)
                nc.tensor.matmul(ps_v1[:], ctk[:, 1152:1280], ctk[:, 512:1024],
                                 start=st, stop=sp)

        # ---------------- attention + injected k-norm ----------------
        ps_sum = psA.tile([1, 512], FP32, tag="bA")
        ps_o = psA.tile([128, 512], FP32, tag="bH")
        ksq = big.tile([128, TNEW], BF16_DT, tag="ksq")
        knw = big.tile([128, TNEW], BF16_DT, tag="knw")
        vsb = big.tile([128, TNEW], BF16_DT, tag="vsb")
        ka = big.tile([128, TNEW], BF16_DT, tag="ka")
        kts_new = big.tile([128, TNEW], BF16_DT, tag="kts_new")
        vt_new = big.tile([128, TNEW], BF16_DT, tag="vt_new")
        rstdT = big.tile([128, 8], FP32, tag="rstdT")

        def knorm_inject(si):
            # k-norm / rope / v-transpose work spread through the old-tile
            # loop; ACT ops land between exps, DVE/PE ops fill idle slots.
            if si == 0:
                nc.scalar.activation(ksq[:, 0:512], ps_k0[:], AF.Square)
                nc.scalar.activation(ksq[:, 512:1024], ps_k1[:], AF.Square)
            elif si == 1:
                nc.scalar.copy(vsb[:, 0:512], ps_v0[:])
                nc.scalar.copy(vsb[:, 512:1024], ps_v1[:])
                nc.vector.tensor_scalar_mul(knw[:, 0:512], ps_k0[:], kw_t[:])
                nc.vector.tensor_scalar_mul(knw[:, 512:1024], ps_k1[:],
                                            kw_t[:])
            elif si == 2:
                ps_sosT = psA.tile([128, 8], FP32, tag="bD")
                for j in range(8):
                    nc.tensor.matmul(ps_sosT[:, j:j + 1],
                                     ksq[:, j * 128:(j + 1) * 128],
                                     ones_colb[:])
                _newton_rsqrt(nc, big, rstdT, ps_sosT, 8, "kn")
            elif si in (3, 4):
                hh = si - 3
                sl = slice(hh * 512, (hh + 1) * 512)
                ps_krot = psA.tile([128, 512], FP32, tag="bE")
                nc.tensor.matmul(ps_krot[:], rotT[:], knw[:, sl])
                nc.vector.tensor_mul(ka[:, sl], knw[:, sl], cosk[:, sl])
                kb = sloop.tile([128, 512], BF16_DT, tag="kb")
                nc.vector.tensor_mul(kb, ps_krot[:], sink[:, sl])
                nc.vector.tensor_add(kts_new[:, sl], ka[:, sl], kb)
            elif 5 <= si <= 12:
                j = si - 5
                ps_vT = psA.tile([128, 128], BF16_DT,
                                 tag=("bD" if j % 2 == 0 else "bE"))
                nc.tensor.transpose(ps_vT[:], vsb[:, j * 128:(j + 1) * 128],
                                    identb[:])
                nc.vector.tensor_copy(vt_new[:, j * 128:(j + 1) * 128],
                                      ps_vT[:])

        with nc.named_scope("sloop"):
            for s in range(ST):
                new = s >= NOLD
                j = s - NOLD
                ps_sc = psA.tile([128, 512], FP32,
                                 tag=("bF" if s % 2 == 0 else "bG"))
                if new:
                    nc.tensor.matmul(ps_sc[:],
                                     kts_new[:, j * 128:(j + 1) * 128],
                                     qT_all[:])
                else:
                    nc.tensor.matmul(ps_sc[:],
                                     kts_old[:, s * 128:(s + 1) * 128],
                                     qT_all[:])
                exr = sloop.tile([128, 512], BF16_DT, tag="exr")
                if new:
                    nc.scalar.activation(exr, ps_sc[:], AF.Exp,
                                         scale=rstdT[:, j:j + 1])
                else:
                    nc.scalar.activation(exr, ps_sc[:], AF.Exp)
                ex = exr
                if mask_all:
                    ex = sloop.tile([128, 512], BF16_DT, tag="exm")
                    nc.vector.tensor_mul(
                        ex, exr, maskF[:, s * 512:(s + 1) * 512])
                elif s == ST - 1:
                    ex = sloop.tile([128, 512], BF16_DT, tag="exm")
                    nc.vector.tensor_mul(ex, exr, mask31[:])
                nc.tensor.matmul(ps_sum[:], ones_colb[:], ex[:],
                                 start=(s == 0), stop=(s == ST - 1))
                vsrc = (vt_new[:, j * 128:(j + 1) * 128] if new
                        else vt_old[:, s * 128:(s + 1) * 128])
                nc.tensor.matmul(ps_o[:], vsrc, ex[:],
                                 start=(s == 0), stop=(s == ST - 1))
                if not new:
                    knorm_inject(s)

        # ---------------- softmax normalize ----------------
        with nc.named_scope("fin"):
            sum_sb = big.tile([1, 512], FP32, tag="sum_sb")
            nc.scalar.copy(sum_sb, ps_sum[:])
            ps_rb = psA.tile([128, 512], FP32, tag="bF")
            nc.tensor.matmul(ps_rb[:], ones_row[:], sum_sb[:])
            rec_sb = big.tile([128, 512], FP32, tag="rec_sb")
            nc.vector.reciprocal(rec_sb, ps_rb[:])
            attT = big.tile([128, 512], BF16_DT, tag="attT")
            nc.vector.tensor_mul(attT, ps_o[:], rec_sb)

        # ---------------- output projection (partial) ----------------
        ytags = ["bA", "bB", "bC", "bD", "bE", "bF", "bG", "bH"]
        with nc.named_scope("oproj"):
            for half in range(2):
                ps_ys = []
                for e in range(half * 4, half * 4 + 4):
                    ps_ys.append(psA.tile([128, 512], FP32, tag=ytags[e],
                                          name=f"ps_y{e}"))
                for h in range(REP):
                    for i, e in enumerate(range(half * 4, half * 4 + 4)):
                        nc.tensor.matmul(
                            ps_ys[i][:], attT[:, h * 128:(h + 1) * 128],
                            woR[:, h * 4096 + e * 512:h * 4096 + (e + 1) * 512],
                            start=(h == 0), stop=(h == REP - 1))
                for i, e in enumerate(range(half * 4, half * 4 + 4)):
                    ysb = ypool.tile([128, 512], FP32, tag=f"ysb{e % 2}")
                    if e % 2 == 0:
                        nc.vector.tensor_copy(ysb, ps_ys[i][:])
                    else:
                        nc.scalar.copy(ysb, ps_ys[i][:])
                    nc.scalar.dma_start(y[:, e * 512:(e + 1) * 512], ysb)


def _prepare_inputs(x, x_ctx, cos_q, sin_q, cos_k, sin_k, kv_cache,
                    causal_mask, Wq, Wk, Wv, Wo, q_norm_w, k_norm_w,
                    mask_all=False):
    """Host-side sharding/preprocessing. Returns list of per-core in_maps."""
    f32 = np.float32
    x = np.asarray(x, f32)
    x_ctx = np.asarray(x_ctx, f32)
    c = np.concatenate([x_ctx[0], x[0]], axis=0)          # [T, HID]
    cT = np.ascontiguousarray(c.T).astype(BF16)           # [HID, T]

    # x.T packed [p, (k l)]: xTp[p, k*128+l] = c.T[k*128+p, T-L+l]
    xTp = np.ascontiguousarray(
        c.T[:, T - L:T].reshape(KT, 128, L).transpose(1, 0, 2)
        .reshape(128, KT * L)).astype(BF16)

    m = np.asarray(causal_mask, f32)[0, 0]                # [L, S]
    # multiplicative mask for the last tile: [s_local, (h l)]
    m31 = np.exp(m[:, S - 128:].T)                        # [128, L]
    mask31 = np.ascontiguousarray(
        np.concatenate([m31] * REP, axis=1)).astype(BF16)
    maskF = None
    if mask_all:
        mm = np.exp(m).T.reshape(ST, 128, L)              # [st, s_local, l]
        mF = np.concatenate([mm] * REP, axis=2)           # [st, s_local, 512]
        maskF = np.ascontiguousarray(
            mF.transpose(1, 0, 2).reshape(128, ST * 512)).astype(BF16)

    cosq = np.asarray(cos_q, f32)[0, 0].T * SCALE         # [D, L]
    sinq = np.asarray(sin_q, f32)[0, 0].T * SCALE
    cosq4 = np.ascontiguousarray(np.concatenate([cosq] * REP, axis=1))
    sinq4 = np.ascontiguousarray(np.concatenate([sinq] * REP, axis=1))
    coskT = np.ascontiguousarray(np.asarray(cos_k, f32)[0, 0].T)
    sinkT = np.ascontiguousarray(np.asarray(sin_k, f32)[0, 0].T)

    # rotate-half as a signed permutation: rot(x) = R @ x
    R = np.zeros((128, 128), f32)
    R[:HALF, HALF:] = -np.eye(HALF, dtype=f32)
    R[HALF:, :HALF] = np.eye(HALF, dtype=f32)
    rotT = np.ascontiguousarray(R.T).astype(BF16)

    qwc = np.ascontiguousarray(np.asarray(q_norm_w, f32).reshape(D, 1))
    kwc = np.ascontiguousarray(np.asarray(k_norm_w, f32).reshape(D, 1))

    Wq = np.asarray(Wq, f32)
    Wk = np.asarray(Wk, f32)
    Wv = np.asarray(Wv, f32)
    Wo = np.asarray(Wo, f32)
    kv = np.asarray(kv_cache, f32)

    in_maps = []
    for cidx in range(NCORES):
        hd = slice(cidx * REP * D, (cidx + 1) * REP * D)
        wq_c = Wq[hd].reshape(REP, D, HID)
        wq_c = wq_c - wq_c.mean(axis=1, keepdims=True)    # fold mean-subtract
        wq_c = wq_c.reshape(REP * D, HID)
        wk_c = Wk[cidx * D:(cidx + 1) * D]
        wk_c = wk_c - wk_c.mean(axis=0, keepdims=True)
        wv_c = Wv[cidx * D:(cidx + 1) * D]
        # wq packed [p, (k e)]: wqP[p, k*512+e] = wq_c.T[k*128+p, e]
        wqP = np.ascontiguousarray(
            wq_c.T.reshape(KT, 128, 512).transpose(1, 0, 2)
            .reshape(128, KT * 512)).astype(BF16)
        # ct+wkv packed per hid tile: [p, (k 1280)]
        wkvT = np.concatenate([wk_c.T, wv_c.T], axis=1)   # [HID, 256]
        ct3 = np.empty((KT, 128, 1280), f32)
        ct3[:, :, :1024] = cT.astype(f32).reshape(KT, 128, T)
        ct3[:, :, 1024:] = wkvT.reshape(KT, 128, 256)
        ctPk = np.ascontiguousarray(
            ct3.transpose(1, 0, 2).reshape(128, KT * 1280)).astype(BF16)
        # wo packed [p, (h e j)]: woP[p, h*4096+e*512+j] = Wo.T[h*128+p, e*512+j]
        woTc = Wo[:, hd].T.reshape(REP, 128, HID)
        woPk = np.ascontiguousarray(
            woTc.transpose(1, 0, 2).reshape(128, REP * HID)).astype(BF16)
        ktold = np.ascontiguousarray(kv[0, cidx, T:, :].T)  # [D, SOLD]
        voldP = np.ascontiguousarray(
            kv[1, cidx, T:, :].reshape(SOLD // 128, 128, D)
            .transpose(1, 0, 2).reshape(128, SOLD))
        im = dict(
            wqP=wqP, ctP=ctPk, woP=woPk, xTp=xTp,
            ktold=ktold.astype(BF16), voldP=voldP.astype(BF16),
            identb2=np.eye(128, dtype=f32).astype(BF16),
            rotTd=rotT,
            cosq4d=cosq4.astype(BF16), sinq4d=sinq4.astype(f32),
            coskd=coskT.astype(BF16), sinkd=sinkT.astype(f32),
            qwd=qwc, kwd=kwc, mask31d=mask31,
        )
        if mask_all:
            im["maskFd"] = maskF
        in_maps.append(im)
    return in_maps


def kernel(**inputs) -> np.ndarray:
    global LAST_RESULTS
    m = np.asarray(inputs["causal_mask"], np.float32)[0, 0]
    mask_all = not bool(np.all(m[:, :S - L] == 0.0))
    key = f"nc_{int(mask_all)}"
    if key not in _PROGRAM_CACHE:
        _PROGRAM_CACHE[key] = _build_program(mask_all=mask_all)
    nc = _PROGRAM_CACHE[key]
    in_maps = _prepare_inputs(**inputs, mask_all=mask_all)
    trace = bool(int(os.environ.get("BASS_KERNEL_TRACE", "0")))
    res = run_bass_kernel_spmd(nc, in_maps, list(range(NCORES)), trace=trace)
    LAST_RESULTS = res
    yacc = np.zeros((L, HID), np.float64)
    for cidx in range(NCORES):
        yacc += res.results[cidx]["y"].astype(np.float64)
    return yacc.astype(np.float32).reshape(1, L, HID)
